# revision 14
# baseline (speedup 1.0000x reference)
"""Trainium2 Bass kernel for CAAN cross-asset attention.

Reference computation (per batch b of 8):
    q = x @ Wq + bq ; k = x @ Wk + bk ; v = x @ Wv + bv
    beta = softmax((q @ k^T) / 16, axis=-1)          # (N, N), N=2048
    out  = (beta @ v) @ Ww + bw                      # (N,)

Algebraic folding used here (exact up to fp error; ~1e-3 rel err with bf16
matmul operands):
    A   = Wq @ Wk^T                      (host fold, f64->f32)
    s[n,m]  = x_n A x_m^T + row_const[n] + x_m.(Wk bq)
    row-constant terms cancel in softmax, so with
    G = x @ A,  beta16[m] = x_m.(Wk bq)/16,  u0[m] = x_m.(Wv Ww):
    p[n,m]  = exp((G_n . x_m)/16 + beta16[m])
    out[n]  = (sum_m p[n,m] u0[m]) / (sum_m p[n,m]) + (bw + bv.Ww)

Device kernel (SPMD, 1 batch element per NeuronCore, 8 cores):
    - inputs passed as bf16 from host; xT loaded via DMA-transpose (xbar)
    - GT = A-projection of xT (TensorE, bf16 in / f32 PSUM accum)
    - main loop per 1024-wide n-block, per 128-wide m-chunk:
        sT = xT_chunk^T @ GT        (scores, transposed layout, f32 PSUM)
        p  = exp(sT/16 + beta16)    (ScalarE, per-partition bias, bf16 out)
        [numer; denom] += [u0,1]^T @ p   (TensorE, M=2, f32 PSUM accum)
      (u0/beta16 column pairs are computed inline during the first n-block,
       reusing the score matmuls' stationary xT slices)
    - DMA [numer; denom] out; final tiny divide + bw_eff on host
"""

import numpy as np
import ml_dtypes
from contextlib import ExitStack

import concourse.bass as bass
import concourse.tile as tile
from concourse import bacc, mybir
from concourse.bass_utils import run_bass_kernel_spmd

N = 2048          # assets per batch element
H = 256           # hidden
NCORES = 8
P = 128           # partitions
HC = H // P       # h chunks (2)
MC = N // P       # m chunks (16)
NBS = 1024        # n block size
NB = N // NBS     # n blocks (2)

F32 = mybir.dt.float32
BF16 = mybir.dt.bfloat16
EXP = mybir.ActivationFunctionType.Exp


def _kernel_body(ctx: ExitStack, tc: "tile.TileContext", out_ap, x_ap, a_ap, w2_ap):
    nc = tc.nc

    singles = ctx.enter_context(tc.tile_pool(name="singles", bufs=1))

    # xT[p, hc, n] = x[n, hc*128+p] via xbar DMA-transpose (bf16 DRAM source).
    # Two transposes over n-halves so downstream matmuls can start on half 0.
    xT = singles.tile([P, HC, N], BF16)
    for half in range(2):
        nc.sync.dma_start_transpose(
            xT[:, :, half * (N // 2):(half + 1) * (N // 2)],
            x_ap[half * (N // 2):(half + 1) * (N // 2), :],
        )

    # A weights: a_sb[p, ic, o] = A[ic*128+p, o]
    a_sb = singles.tile([P, HC, H], BF16)
    nc.sync.dma_start(out=a_sb, in_=a_ap.rearrange("(c p) o -> p c o", p=P))
    # W2 = [Wv@Ww | (Wk@bq)/16]: w2_sb[p, ic, c]
    w2_sb = singles.tile([P, HC, 2], BF16)
    nc.sync.dma_start(out=w2_sb, in_=w2_ap.rearrange("(c p) o -> p c o", p=P))

    # GT[p, oc, n] = (x@A)[n, oc*128+p]
    GT = singles.tile([P, HC, N], BF16)
    # UBb[p, c, j]: c=0 -> u0 (bf16), c=2 -> ones; UBf[p, j]: beta16 (f32 bias)
    UBb = singles.tile([P, 3, MC], BF16)
    UBf = singles.tile([P, MC], F32)
    nc.vector.memset(UBb[:, 2, :], 1.0)

    # ---- Phase 1: GT projection + UB columns (pools released before main loop) ----
    with (
        tc.tile_pool(name="gp", bufs=1, space="PSUM") as gpp,
        tc.tile_pool(name="up", bufs=4, space="PSUM") as upp,
    ):
        for oc in range(HC):
            gs = [gpp.tile([P, 512], F32, name=f"g{ns}", tag=f"g{ns}") for ns in range(4)]
            for ic in range(HC):
                for ns in range(4):
                    nc.tensor.matmul(
                        gs[ns],
                        a_sb[:, ic, oc * 128:(oc + 1) * 128],
                        xT[:, ic, ns * 512:(ns + 1) * 512],
                        start=(ic == 0),
                        stop=(ic == HC - 1),
                    )
            for ns in range(4):
                nc.vector.tensor_copy(GT[:, oc, ns * 512:(ns + 1) * 512], gs[ns])
        for j in range(MC):
            ub = upp.tile([P, 2], F32)
            for ic in range(HC):
                nc.tensor.matmul(
                    ub,
                    xT[:, ic, j * 128:(j + 1) * 128],
                    w2_sb[:, ic, :],
                    start=(ic == 0),
                    stop=(ic == HC - 1),
                )
            nc.vector.tensor_copy(UBb[:, 0:1, j], ub[:, 0:1])
            nc.vector.tensor_copy(UBf[:, j:j + 1], ub[:, 1:2])

    # ---- Phase 2: main attention loop ----
    ppool = ctx.enter_context(tc.tile_pool(name="pexp", bufs=3))
    spool = ctx.enter_context(tc.tile_pool(name="spsum", bufs=2, space="PSUM"))
    ndpool = ctx.enter_context(tc.tile_pool(name="ndpsum", bufs=1, space="PSUM"))
    fin = ctx.enter_context(tc.tile_pool(name="fin", bufs=2))

    nd = ndpool.tile([2, N], F32)  # row 0: numer, row 1: denom

    for nb in range(NB):
        for j in range(MC):
            sT = spool.tile([P, NBS], F32)
            for ic in range(HC):
                # scores: stationary xT[:, ic, j-chunk], stream GT
                for s in range(NBS // 512):
                    nc.tensor.matmul(
                        sT[:, s * 512:(s + 1) * 512],
                        xT[:, ic, j * 128:(j + 1) * 128],
                        GT[:, ic, nb * NBS + s * 512: nb * NBS + (s + 1) * 512],
                        start=(ic == 0),
                        stop=(ic == HC - 1),
                    )
            p = ppool.tile([P, NBS], BF16)
            nc.scalar.activation(p, sT, EXP, bias=UBf[:, j:j + 1], scale=0.0625)
            for s in range(NBS // 512):
                nc.tensor.matmul(
                    nd[:, nb * NBS + s * 512: nb * NBS + (s + 1) * 512],
                    UBb[:, 0::2, j],
                    p[:, s * 512:(s + 1) * 512],
                    start=(j == 0),
                    stop=(j == MC - 1),
                )

        # evacuate this n-block's [numer; denom] (final divide happens on host)
        ob = fin.tile([2, NBS], F32, tag="ob")
        nc.vector.tensor_copy(ob, nd[:, nb * NBS:(nb + 1) * NBS])
        nc.sync.dma_start(out_ap[:, nb * NBS:(nb + 1) * NBS], ob)


def build_program():
    nc = bacc.Bacc("TRN2", target_bir_lowering=False, debug=False)
    x_ap = nc.dram_tensor("x", [N, H], BF16, kind="ExternalInput").ap()
    a_ap = nc.dram_tensor("wa", [H, H], BF16, kind="ExternalInput").ap()
    w2_ap = nc.dram_tensor("w2", [H, 2], BF16, kind="ExternalInput").ap()
    out_ap = nc.dram_tensor("out", [2, N], F32, kind="ExternalOutput").ap()
    with tile.TileContext(nc) as tc:
        with ExitStack() as ctx:
            _kernel_body(ctx, tc, out_ap, x_ap, a_ap, w2_ap)
    nc.compile()
    return nc


_PROGRAM = None


def _get_program():
    global _PROGRAM
    if _PROGRAM is None:
        _PROGRAM = build_program()
    return _PROGRAM


def host_fold(Wq, bq, Wk, bk, Wv, bv, Ww, bw):
    """Fold the projection weights (f64 accumulate, f32 store)."""
    A = (Wq.astype(np.float64) @ Wk.astype(np.float64).T).astype(np.float32)
    b16 = ((Wk.astype(np.float64) @ bq.astype(np.float64)) / 16.0).astype(np.float32)
    wvp = (Wv.astype(np.float64) @ Ww.astype(np.float64)[:, 0]).astype(np.float32)
    W2 = np.stack([wvp, b16], axis=1).astype(np.float32)  # [H, 2]
    bw_eff = np.float32(bw[0] + bv.astype(np.float64) @ Ww.astype(np.float64)[:, 0])
    return A, W2, bw_eff


def run(x, Wq, bq, Wk, bk, Wv, bv, Ww, bw, trace=False):
    """Returns (out [8, N], BassKernelResults)."""
    x = np.asarray(x, dtype=np.float32)
    A, W2, bw_eff = host_fold(
        np.asarray(Wq), np.asarray(bq), np.asarray(Wk), np.asarray(bk),
        np.asarray(Wv), np.asarray(bv), np.asarray(Ww), np.asarray(bw),
    )
    x16 = np.ascontiguousarray(x.astype(ml_dtypes.bfloat16))
    A16 = np.ascontiguousarray(A.astype(ml_dtypes.bfloat16))
    W216 = np.ascontiguousarray(W2.astype(ml_dtypes.bfloat16))

    nc = _get_program()
    in_maps = [
        {"x": x16[b], "wa": A16, "w2": W216}
        for b in range(NCORES)
    ]
    res = run_bass_kernel_spmd(nc, in_maps, list(range(NCORES)), trace=trace)
    out = np.stack(
        [res.results[b]["out"][0] / res.results[b]["out"][1] + bw_eff for b in range(NCORES)],
        axis=0,
    )
    return out.astype(np.float32), res


def kernel(x, Wq, bq, Wk, bk, Wv, bv, Ww, bw):
    out, _ = run(x, Wq, bq, Wk, bk, Wv, bv, Ww, bw)
    return out


if __name__ == "__main__":
    rng = np.random.default_rng(0)
    s = 1.0 / np.sqrt(H)
    inputs = {
        "x": rng.standard_normal((8, N, H), dtype=np.float32),
        "Wq": rng.uniform(-s, s, (H, H)).astype(np.float32),
        "bq": rng.uniform(-s, s, (H,)).astype(np.float32),
        "Wk": rng.uniform(-s, s, (H, H)).astype(np.float32),
        "bk": rng.uniform(-s, s, (H,)).astype(np.float32),
        "Wv": rng.uniform(-s, s, (H, H)).astype(np.float32),
        "bv": rng.uniform(-s, s, (H,)).astype(np.float32),
        "Ww": rng.uniform(-s, s, (H, 1)).astype(np.float32),
        "bw": rng.uniform(-s, s, (1,)).astype(np.float32),
    }
    out = kernel(**inputs)
    print("kernel out:", out.shape, out.dtype, out[0, :4])


# revision 16
# speedup vs baseline: 1.0904x; 1.0904x over previous
"""Trainium2 Bass kernel for CAAN cross-asset attention.

Reference computation (per batch b of 8):
    q = x @ Wq + bq ; k = x @ Wk + bk ; v = x @ Wv + bv
    beta = softmax((q @ k^T) / 16, axis=-1)          # (N, N), N=2048
    out  = (beta @ v) @ Ww + bw                      # (N,)

Algebraic folding used here (exact up to fp error; ~1e-3 rel err with bf16
matmul operands):
    A   = Wq @ Wk^T                      (host fold, f64->f32)
    s[n,m]  = x_n A x_m^T + row_const[n] + x_m.(Wk bq)
    row-constant terms cancel in softmax, so with
    G = x @ A,  beta16[m] = x_m.(Wk bq)/16,  u0[m] = x_m.(Wv Ww):
    p[n,m]  = exp((G_n . x_m)/16 + beta16[m])
    out[n]  = (sum_m p[n,m] u0[m]) / (sum_m p[n,m]) + (bw + bv.Ww)

Device kernel (SPMD, 1 batch element per NeuronCore, 8 cores):
    - inputs passed as bf16 from host; xT loaded via DMA-transpose (xbar)
    - GT = A-projection of xT (TensorE, bf16 in / f32 PSUM accum)
    - main loop per 1024-wide n-block, per 128-wide m-chunk:
        sT = xT_chunk^T @ GT        (scores, transposed layout, f32 PSUM)
        p  = exp(sT/16 + beta16)    (ScalarE, per-partition bias, bf16 out)
        [numer; denom] += [u0,1]^T @ p   (TensorE, M=2, f32 PSUM accum)
      (u0/beta16 column pairs are computed inline during the first n-block,
       reusing the score matmuls' stationary xT slices)
    - DMA [numer; denom] out; final tiny divide + bw_eff on host
"""

import numpy as np
import ml_dtypes
from contextlib import ExitStack

import concourse.bass as bass
import concourse.tile as tile
from concourse import bacc, mybir
from concourse.bass_utils import run_bass_kernel_spmd

N = 2048          # assets per batch element
H = 256           # hidden
NCORES = 8
P = 128           # partitions
HC = H // P       # h chunks (2)
MC = N // P       # m chunks (16)
NBS = 1024        # n block size
NB = N // NBS     # n blocks (2)

F32 = mybir.dt.float32
BF16 = mybir.dt.bfloat16
EXP = mybir.ActivationFunctionType.Exp


def _kernel_body(ctx: ExitStack, tc: "tile.TileContext", out_ap, x_ap, a_ap, w2_ap):
    nc = tc.nc

    singles = ctx.enter_context(tc.tile_pool(name="singles", bufs=1))

    # xT[p, hc, n] = x[n, hc*128+p] via xbar DMA-transpose (bf16 DRAM source).
    # Two transposes over n-halves so downstream matmuls can start on half 0.
    xT = singles.tile([P, HC, N], BF16)
    for half in range(2):
        nc.sync.dma_start_transpose(
            xT[:, :, half * (N // 2):(half + 1) * (N // 2)],
            x_ap[half * (N // 2):(half + 1) * (N // 2), :],
        )

    # A weights: a_sb[p, ic, o] = A[ic*128+p, o]
    a_sb = singles.tile([P, HC, H], BF16)
    nc.sync.dma_start(out=a_sb, in_=a_ap.rearrange("(c p) o -> p c o", p=P))
    # W2 = [Wv@Ww | (Wk@bq)/16]: w2_sb[p, ic, c]
    w2_sb = singles.tile([P, HC, 2], BF16)
    nc.sync.dma_start(out=w2_sb, in_=w2_ap.rearrange("(c p) o -> p c o", p=P))

    # GT[p, oc, n] = (x@A)[n, oc*128+p]
    GT = singles.tile([P, HC, N], BF16)
    # UBb[p, c, j]: c=0 -> u0 (bf16), c=2 -> ones; UBf[p, j]: beta16 (f32 bias)
    UBb = singles.tile([P, 3, MC], BF16)
    UBf = singles.tile([P, MC], F32)
    nc.vector.memset(UBb[:, 2, :], 1.0)

    # ---- Phase 1: GT projection + UB columns (pools released before main loop) ----
    with (
        tc.tile_pool(name="gp", bufs=1, space="PSUM") as gpp,
        tc.tile_pool(name="up", bufs=4, space="PSUM") as upp,
    ):
        for oc in range(HC):
            gs = [gpp.tile([P, 512], F32, name=f"g{ns}", tag=f"g{ns}") for ns in range(4)]
            for ic in range(HC):
                for ns in range(4):
                    nc.tensor.matmul(
                        gs[ns],
                        a_sb[:, ic, oc * 128:(oc + 1) * 128],
                        xT[:, ic, ns * 512:(ns + 1) * 512],
                        start=(ic == 0),
                        stop=(ic == HC - 1),
                    )
            for ns in range(4):
                nc.vector.tensor_copy(GT[:, oc, ns * 512:(ns + 1) * 512], gs[ns])
        for j in range(MC):
            ub = upp.tile([P, 2], F32)
            for ic in range(HC):
                nc.tensor.matmul(
                    ub,
                    xT[:, ic, j * 128:(j + 1) * 128],
                    w2_sb[:, ic, :],
                    start=(ic == 0),
                    stop=(ic == HC - 1),
                )
            nc.vector.tensor_copy(UBb[:, 0:1, j], ub[:, 0:1])
            nc.vector.tensor_copy(UBf[:, j:j + 1], ub[:, 1:2])

    # ---- Phase 2: main attention loop ----
    ppool = ctx.enter_context(tc.tile_pool(name="pexp", bufs=3))
    spool = ctx.enter_context(tc.tile_pool(name="spsum", bufs=3, space="PSUM"))
    ndpool = ctx.enter_context(tc.tile_pool(name="ndpsum", bufs=1, space="PSUM"))
    fin = ctx.enter_context(tc.tile_pool(name="fin", bufs=1))

    # [numer; denom] packed into ONE PSUM bank: region (nb, s) sits at
    # partition base 32*(nb*2+s), rows +0 (numer) / +1 (denom), via col-group
    # tile_position. Host reassembles.
    nd4 = ndpool.tile([P, 512], F32)
    nc.vector.memset(nd4, 0.0)

    for nb in range(NB):
        for j in range(MC):
            sT = spool.tile([P, NBS], F32)
            for ic in range(HC):
                # scores: stationary xT[:, ic, j-chunk], stream GT
                for s in range(NBS // 512):
                    nc.tensor.matmul(
                        sT[:, s * 512:(s + 1) * 512],
                        xT[:, ic, j * 128:(j + 1) * 128],
                        GT[:, ic, nb * NBS + s * 512: nb * NBS + (s + 1) * 512],
                        start=(ic == 0),
                        stop=(ic == HC - 1),
                    )
            p = ppool.tile([P, NBS], BF16)
            nc.scalar.activation(p, sT, EXP, bias=UBf[:, j:j + 1], scale=0.0625)
            for s in range(NBS // 512):
                base = 32 * (nb * 2 + s)
                nc.tensor.matmul(
                    nd4[base:base + 2, :],
                    UBb[:, 0::2, j],
                    p[:, s * 512:(s + 1) * 512],
                    start=(j == 0),
                    stop=(j == MC - 1),
                    tile_position=(0, base),
                )

    # evacuate packed [numer; denom] (host divides + reassembles)
    ob = fin.tile([P, 512], F32)
    nc.vector.tensor_copy(ob, nd4)
    nc.sync.dma_start(out_ap, ob)


def build_program():
    nc = bacc.Bacc("TRN2", target_bir_lowering=False, debug=False)
    x_ap = nc.dram_tensor("x", [N, H], BF16, kind="ExternalInput").ap()
    a_ap = nc.dram_tensor("wa", [H, H], BF16, kind="ExternalInput").ap()
    w2_ap = nc.dram_tensor("w2", [H, 2], BF16, kind="ExternalInput").ap()
    out_ap = nc.dram_tensor("out", [P, 512], F32, kind="ExternalOutput").ap()
    with tile.TileContext(nc) as tc:
        with ExitStack() as ctx:
            _kernel_body(ctx, tc, out_ap, x_ap, a_ap, w2_ap)
    nc.compile()
    return nc


_PROGRAM = None


def _get_program():
    global _PROGRAM
    if _PROGRAM is None:
        _PROGRAM = build_program()
    return _PROGRAM


def host_fold(Wq, bq, Wk, bk, Wv, bv, Ww, bw):
    """Fold the projection weights (f64 accumulate, f32 store)."""
    A = (Wq.astype(np.float64) @ Wk.astype(np.float64).T).astype(np.float32)
    b16 = ((Wk.astype(np.float64) @ bq.astype(np.float64)) / 16.0).astype(np.float32)
    wvp = (Wv.astype(np.float64) @ Ww.astype(np.float64)[:, 0]).astype(np.float32)
    W2 = np.stack([wvp, b16], axis=1).astype(np.float32)  # [H, 2]
    bw_eff = np.float32(bw[0] + bv.astype(np.float64) @ Ww.astype(np.float64)[:, 0])
    return A, W2, bw_eff


def run(x, Wq, bq, Wk, bk, Wv, bv, Ww, bw, trace=False):
    """Returns (out [8, N], BassKernelResults)."""
    x = np.asarray(x, dtype=np.float32)
    A, W2, bw_eff = host_fold(
        np.asarray(Wq), np.asarray(bq), np.asarray(Wk), np.asarray(bk),
        np.asarray(Wv), np.asarray(bv), np.asarray(Ww), np.asarray(bw),
    )
    x16 = np.ascontiguousarray(x.astype(ml_dtypes.bfloat16))
    A16 = np.ascontiguousarray(A.astype(ml_dtypes.bfloat16))
    W216 = np.ascontiguousarray(W2.astype(ml_dtypes.bfloat16))

    nc = _get_program()
    in_maps = [
        {"x": x16[b], "wa": A16, "w2": W216}
        for b in range(NCORES)
    ]
    res = run_bass_kernel_spmd(nc, in_maps, list(range(NCORES)), trace=trace)

    def _final(o):
        numer = np.concatenate([o[0], o[32], o[64], o[96]])
        denom = np.concatenate([o[1], o[33], o[65], o[97]])
        return numer / denom + bw_eff

    out = np.stack([_final(res.results[b]["out"]) for b in range(NCORES)], axis=0)
    return out.astype(np.float32), res


def kernel(x, Wq, bq, Wk, bk, Wv, bv, Ww, bw):
    out, _ = run(x, Wq, bq, Wk, bk, Wv, bv, Ww, bw)
    return out


if __name__ == "__main__":
    rng = np.random.default_rng(0)
    s = 1.0 / np.sqrt(H)
    inputs = {
        "x": rng.standard_normal((8, N, H), dtype=np.float32),
        "Wq": rng.uniform(-s, s, (H, H)).astype(np.float32),
        "bq": rng.uniform(-s, s, (H,)).astype(np.float32),
        "Wk": rng.uniform(-s, s, (H, H)).astype(np.float32),
        "bk": rng.uniform(-s, s, (H,)).astype(np.float32),
        "Wv": rng.uniform(-s, s, (H, H)).astype(np.float32),
        "bv": rng.uniform(-s, s, (H,)).astype(np.float32),
        "Ww": rng.uniform(-s, s, (H, 1)).astype(np.float32),
        "bw": rng.uniform(-s, s, (1,)).astype(np.float32),
    }
    out = kernel(**inputs)
    print("kernel out:", out.shape, out.dtype, out[0, :4])


# revision 17
# speedup vs baseline: 1.1778x; 1.0802x over previous
"""Trainium2 Bass kernel for CAAN cross-asset attention.

Reference computation (per batch b of 8):
    q = x @ Wq + bq ; k = x @ Wk + bk ; v = x @ Wv + bv
    beta = softmax((q @ k^T) / 16, axis=-1)          # (N, N), N=2048
    out  = (beta @ v) @ Ww + bw                      # (N,)

Algebraic folding used here (exact up to fp error; ~1e-3 rel err with bf16
matmul operands):
    A   = Wq @ Wk^T                      (host fold, f64->f32)
    s[n,m]  = x_n A x_m^T + row_const[n] + x_m.(Wk bq)
    row-constant terms cancel in softmax, so with
    G = x @ A,  beta16[m] = x_m.(Wk bq)/16,  u0[m] = x_m.(Wv Ww):
    p[n,m]  = exp((G_n . x_m)/16 + beta16[m])
    out[n]  = (sum_m p[n,m] u0[m]) / (sum_m p[n,m]) + (bw + bv.Ww)

Device kernel (SPMD, 1 batch element per NeuronCore, 8 cores):
    - inputs passed as bf16 from host; xT loaded via DMA-transpose (xbar)
    - GT = A-projection of xT (TensorE, bf16 in / f32 PSUM accum)
    - main loop per 1024-wide n-block, per 128-wide m-chunk:
        sT = xT_chunk^T @ GT        (scores, transposed layout, f32 PSUM)
        p  = exp(sT/16 + beta16)    (ScalarE, per-partition bias, bf16 out)
        [numer; denom] += [u0,1]^T @ p   (TensorE, M=2, f32 PSUM accum)
      (u0/beta16 column pairs are computed inline during the first n-block,
       reusing the score matmuls' stationary xT slices)
    - DMA [numer; denom] out; final tiny divide + bw_eff on host
"""

import numpy as np
import ml_dtypes
from contextlib import ExitStack

import concourse.bass as bass
import concourse.tile as tile
from concourse import bacc, mybir
from concourse.bass_utils import run_bass_kernel_spmd

N = 2048          # assets per batch element
H = 256           # hidden
NCORES = 8
P = 128           # partitions
HC = H // P       # h chunks (2)
MC = N // P       # m chunks (16)
NBS = 1024        # n block size
NB = N // NBS     # n blocks (2)

F32 = mybir.dt.float32
BF16 = mybir.dt.bfloat16
EXP = mybir.ActivationFunctionType.Exp


def _kernel_body(ctx: ExitStack, tc: "tile.TileContext", out_ap, x_ap, a_ap, w2_ap):
    nc = tc.nc

    singles = ctx.enter_context(tc.tile_pool(name="singles", bufs=1))

    # A weights: a_sb[p, ic, o] = A[ic*128+p, o]
    a_sb = singles.tile([P, HC, H], BF16)
    nc.sync.dma_start(out=a_sb, in_=a_ap.rearrange("(c p) o -> p c o", p=P))
    # W2 = [Wv@Ww | (Wk@bq)/16]: w2_sb[p, ic, c]
    w2_sb = singles.tile([P, HC, 2], BF16)
    nc.sync.dma_start(out=w2_sb, in_=w2_ap.rearrange("(c p) o -> p c o", p=P))

    # xT[p, hc, n] = x[n, hc*128+p]; host supplies x already transposed
    # as [H, N] bf16, so this is a plain fast DMA.
    xT = singles.tile([P, HC, N], BF16)
    nc.sync.dma_start(out=xT, in_=x_ap.rearrange("(c p) n -> p c n", p=P))

    # GT[p, oc, n] = (x@A)[n, oc*128+p]
    GT = singles.tile([P, HC, N], BF16)
    # UBb[p, c, j]: c=0 -> u0 (bf16), c=2 -> ones; UBf[p, j]: beta16 (f32 bias)
    UBb = singles.tile([P, 3, MC], BF16)
    UBf = singles.tile([P, MC], F32)
    nc.vector.memset(UBb[:, 2, :], 1.0)

    # ---- Phase 1: GT projection + UB columns (pools released before main loop) ----
    with (
        tc.tile_pool(name="gp", bufs=1, space="PSUM") as gpp,
        tc.tile_pool(name="up", bufs=4, space="PSUM") as upp,
    ):
        for oc in range(HC):
            gs = [gpp.tile([P, 512], F32, name=f"g{ns}", tag=f"g{ns}") for ns in range(4)]
            for ic in range(HC):
                for ns in range(4):
                    nc.tensor.matmul(
                        gs[ns],
                        a_sb[:, ic, oc * 128:(oc + 1) * 128],
                        xT[:, ic, ns * 512:(ns + 1) * 512],
                        start=(ic == 0),
                        stop=(ic == HC - 1),
                    )
            for ns in range(4):
                nc.vector.tensor_copy(GT[:, oc, ns * 512:(ns + 1) * 512], gs[ns])
        for j in range(MC):
            ub = upp.tile([P, 2], F32)
            for ic in range(HC):
                nc.tensor.matmul(
                    ub,
                    xT[:, ic, j * 128:(j + 1) * 128],
                    w2_sb[:, ic, :],
                    start=(ic == 0),
                    stop=(ic == HC - 1),
                )
            nc.vector.tensor_copy(UBb[:, 0:1, j], ub[:, 0:1])
            nc.vector.tensor_copy(UBf[:, j:j + 1], ub[:, 1:2])

    # ---- Phase 2: main attention loop ----
    ppool = ctx.enter_context(tc.tile_pool(name="pexp", bufs=4))
    spool = ctx.enter_context(tc.tile_pool(name="spsum", bufs=3, space="PSUM"))
    ndpool = ctx.enter_context(tc.tile_pool(name="ndpsum", bufs=1, space="PSUM"))
    fin = ctx.enter_context(tc.tile_pool(name="fin", bufs=1))

    # [numer; denom] packed into ONE PSUM bank: region (nb, s) sits at
    # partition base 32*(nb*2+s), rows +0 (numer) / +1 (denom), via col-group
    # tile_position. Host reassembles.
    nd4 = ndpool.tile([P, 512], F32)
    nc.vector.memset(nd4, 0.0)

    for nb in range(NB):
        for j in range(MC):
            sT = spool.tile([P, NBS], F32)
            for ic in range(HC):
                # scores: stationary xT[:, ic, j-chunk], stream GT
                for s in range(NBS // 512):
                    nc.tensor.matmul(
                        sT[:, s * 512:(s + 1) * 512],
                        xT[:, ic, j * 128:(j + 1) * 128],
                        GT[:, ic, nb * NBS + s * 512: nb * NBS + (s + 1) * 512],
                        start=(ic == 0),
                        stop=(ic == HC - 1),
                    )
            p = ppool.tile([P, NBS], BF16)
            nc.scalar.activation(p, sT, EXP, bias=UBf[:, j:j + 1], scale=0.0625)
            for s in range(NBS // 512):
                base = 32 * (nb * 2 + s)
                nc.tensor.matmul(
                    nd4[base:base + 2, :],
                    UBb[:, 0::2, j],
                    p[:, s * 512:(s + 1) * 512],
                    start=(j == 0),
                    stop=(j == MC - 1),
                    tile_position=(0, base),
                )

    # evacuate packed [numer; denom] (host divides + reassembles)
    ob = fin.tile([P, 512], F32)
    nc.vector.tensor_copy(ob, nd4)
    nc.sync.dma_start(out_ap, ob)


def build_program():
    nc = bacc.Bacc("TRN2", target_bir_lowering=False, debug=False)
    x_ap = nc.dram_tensor("x", [H, N], BF16, kind="ExternalInput").ap()
    a_ap = nc.dram_tensor("wa", [H, H], BF16, kind="ExternalInput").ap()
    w2_ap = nc.dram_tensor("w2", [H, 2], BF16, kind="ExternalInput").ap()
    out_ap = nc.dram_tensor("out", [P, 512], F32, kind="ExternalOutput").ap()
    with tile.TileContext(nc) as tc:
        with ExitStack() as ctx:
            _kernel_body(ctx, tc, out_ap, x_ap, a_ap, w2_ap)
    nc.compile()
    return nc


_PROGRAM = None


def _get_program():
    global _PROGRAM
    if _PROGRAM is None:
        _PROGRAM = build_program()
    return _PROGRAM


def host_fold(Wq, bq, Wk, bk, Wv, bv, Ww, bw):
    """Fold the projection weights (f64 accumulate, f32 store)."""
    A = (Wq.astype(np.float64) @ Wk.astype(np.float64).T).astype(np.float32)
    b16 = ((Wk.astype(np.float64) @ bq.astype(np.float64)) / 16.0).astype(np.float32)
    wvp = (Wv.astype(np.float64) @ Ww.astype(np.float64)[:, 0]).astype(np.float32)
    W2 = np.stack([wvp, b16], axis=1).astype(np.float32)  # [H, 2]
    bw_eff = np.float32(bw[0] + bv.astype(np.float64) @ Ww.astype(np.float64)[:, 0])
    return A, W2, bw_eff


def run(x, Wq, bq, Wk, bk, Wv, bv, Ww, bw, trace=False):
    """Returns (out [8, N], BassKernelResults)."""
    x = np.asarray(x, dtype=np.float32)
    A, W2, bw_eff = host_fold(
        np.asarray(Wq), np.asarray(bq), np.asarray(Wk), np.asarray(bk),
        np.asarray(Wv), np.asarray(bv), np.asarray(Ww), np.asarray(bw),
    )
    # pre-transpose per batch: [B, N, H] -> [B, H, N] (bf16)
    x16 = np.ascontiguousarray(x.astype(ml_dtypes.bfloat16).transpose(0, 2, 1))
    A16 = np.ascontiguousarray(A.astype(ml_dtypes.bfloat16))
    W216 = np.ascontiguousarray(W2.astype(ml_dtypes.bfloat16))

    nc = _get_program()
    in_maps = [
        {"x": x16[b], "wa": A16, "w2": W216}
        for b in range(NCORES)
    ]
    res = run_bass_kernel_spmd(nc, in_maps, list(range(NCORES)), trace=trace)

    def _final(o):
        numer = np.concatenate([o[0], o[32], o[64], o[96]])
        denom = np.concatenate([o[1], o[33], o[65], o[97]])
        return numer / denom + bw_eff

    out = np.stack([_final(res.results[b]["out"]) for b in range(NCORES)], axis=0)
    return out.astype(np.float32), res


def kernel(x, Wq, bq, Wk, bk, Wv, bv, Ww, bw):
    out, _ = run(x, Wq, bq, Wk, bk, Wv, bv, Ww, bw)
    return out


if __name__ == "__main__":
    rng = np.random.default_rng(0)
    s = 1.0 / np.sqrt(H)
    inputs = {
        "x": rng.standard_normal((8, N, H), dtype=np.float32),
        "Wq": rng.uniform(-s, s, (H, H)).astype(np.float32),
        "bq": rng.uniform(-s, s, (H,)).astype(np.float32),
        "Wk": rng.uniform(-s, s, (H, H)).astype(np.float32),
        "bk": rng.uniform(-s, s, (H,)).astype(np.float32),
        "Wv": rng.uniform(-s, s, (H, H)).astype(np.float32),
        "bv": rng.uniform(-s, s, (H,)).astype(np.float32),
        "Ww": rng.uniform(-s, s, (H, 1)).astype(np.float32),
        "bw": rng.uniform(-s, s, (1,)).astype(np.float32),
    }
    out = kernel(**inputs)
    print("kernel out:", out.shape, out.dtype, out[0, :4])


# revision 18
# speedup vs baseline: 1.1787x; 1.0008x over previous
"""Trainium2 Bass kernel for CAAN cross-asset attention.

Reference computation (per batch b of 8):
    q = x @ Wq + bq ; k = x @ Wk + bk ; v = x @ Wv + bv
    beta = softmax((q @ k^T) / 16, axis=-1)          # (N, N), N=2048
    out  = (beta @ v) @ Ww + bw                      # (N,)

Algebraic folding used here (exact up to fp error; ~1e-3 rel err with bf16
matmul operands):
    A   = Wq @ Wk^T                      (host fold, f64->f32)
    s[n,m]  = x_n A x_m^T + row_const[n] + x_m.(Wk bq)
    row-constant terms cancel in softmax, so with
    G = x @ A,  beta16[m] = x_m.(Wk bq)/16,  u0[m] = x_m.(Wv Ww):
    p[n,m]  = exp((G_n . x_m)/16 + beta16[m])
    out[n]  = (sum_m p[n,m] u0[m]) / (sum_m p[n,m]) + (bw + bv.Ww)

Device kernel (SPMD, 1 batch element per NeuronCore, 8 cores):
    - inputs passed as bf16 from host; xT loaded via DMA-transpose (xbar)
    - GT = A-projection of xT (TensorE, bf16 in / f32 PSUM accum)
    - main loop per 1024-wide n-block, per 128-wide m-chunk:
        sT = xT_chunk^T @ GT        (scores, transposed layout, f32 PSUM)
        p  = exp(sT/16 + beta16)    (ScalarE, per-partition bias, bf16 out)
        [numer; denom] += [u0,1]^T @ p   (TensorE, M=2, f32 PSUM accum)
      (u0/beta16 column pairs are computed inline during the first n-block,
       reusing the score matmuls' stationary xT slices)
    - DMA [numer; denom] out; final tiny divide + bw_eff on host
"""

import numpy as np
import ml_dtypes
from contextlib import ExitStack

import concourse.bass as bass
import concourse.tile as tile
from concourse import bacc, mybir
from concourse.bass_utils import run_bass_kernel_spmd

N = 2048          # assets per batch element
H = 256           # hidden
NCORES = 8
P = 128           # partitions
HC = H // P       # h chunks (2)
MC = N // P       # m chunks (16)
NBS = 1024        # n block size
NB = N // NBS     # n blocks (2)

F32 = mybir.dt.float32
BF16 = mybir.dt.bfloat16
EXP = mybir.ActivationFunctionType.Exp


def _kernel_body(ctx: ExitStack, tc: "tile.TileContext", out_ap, x_ap, a_ap, w2_ap):
    nc = tc.nc

    singles = ctx.enter_context(tc.tile_pool(name="singles", bufs=1))

    # A weights: a_sb[p, ic, o] = A[ic*128+p, o]
    a_sb = singles.tile([P, HC, H], BF16)
    nc.sync.dma_start(out=a_sb, in_=a_ap.rearrange("(c p) o -> p c o", p=P))
    # W2 = [Wv@Ww | (Wk@bq)/16]: w2_sb[p, ic, c]
    w2_sb = singles.tile([P, HC, 2], BF16)
    nc.sync.dma_start(out=w2_sb, in_=w2_ap.rearrange("(c p) o -> p c o", p=P))

    # xT[p, hc, n] = x[n, hc*128+p]; host supplies x already transposed
    # as [H, N] bf16, so this is a plain fast DMA.
    xT = singles.tile([P, HC, N], BF16)
    x_r = x_ap.rearrange("(c p) n -> p c n", p=P)
    for q in range(4):
        nc.sync.dma_start(out=xT[:, :, q * 512:(q + 1) * 512], in_=x_r[:, :, q * 512:(q + 1) * 512])

    # GT[p, oc, n] = (x@A)[n, oc*128+p]
    GT = singles.tile([P, HC, N], BF16)
    # UBb[p, c, j]: c=0 -> u0 (bf16), c=2 -> ones; UBf[p, j]: beta16 (f32 bias)
    UBb = singles.tile([P, 3, MC], BF16)
    UBf = singles.tile([P, MC], F32)
    nc.vector.memset(UBb[:, 2, :], 1.0)

    # ---- Phase 1: GT projection + UB columns (pools released before main loop) ----
    with (
        tc.tile_pool(name="gp", bufs=1, space="PSUM") as gpp,
        tc.tile_pool(name="up", bufs=4, space="PSUM") as upp,
    ):
        for oc in range(HC):
            gs = [gpp.tile([P, 512], F32, name=f"g{ns}", tag=f"g{ns}") for ns in range(4)]
            for ic in range(HC):
                for ns in range(4):
                    nc.tensor.matmul(
                        gs[ns],
                        a_sb[:, ic, oc * 128:(oc + 1) * 128],
                        xT[:, ic, ns * 512:(ns + 1) * 512],
                        start=(ic == 0),
                        stop=(ic == HC - 1),
                    )
            for ns in range(4):
                nc.vector.tensor_copy(GT[:, oc, ns * 512:(ns + 1) * 512], gs[ns])
        for j in range(MC):
            ub = upp.tile([P, 2], F32)
            for ic in range(HC):
                nc.tensor.matmul(
                    ub,
                    xT[:, ic, j * 128:(j + 1) * 128],
                    w2_sb[:, ic, :],
                    start=(ic == 0),
                    stop=(ic == HC - 1),
                )
            nc.vector.tensor_copy(UBb[:, 0:1, j], ub[:, 0:1])
            nc.vector.tensor_copy(UBf[:, j:j + 1], ub[:, 1:2])

    # ---- Phase 2: main attention loop ----
    ppool = ctx.enter_context(tc.tile_pool(name="pexp", bufs=4))
    spool = ctx.enter_context(tc.tile_pool(name="spsum", bufs=3, space="PSUM"))
    ndpool = ctx.enter_context(tc.tile_pool(name="ndpsum", bufs=1, space="PSUM"))
    fin = ctx.enter_context(tc.tile_pool(name="fin", bufs=1))

    # [numer; denom] packed into ONE PSUM bank: region (nb, s) sits at
    # partition base 32*(nb*2+s), rows +0 (numer) / +1 (denom), via col-group
    # tile_position. Host reassembles.
    nd4 = ndpool.tile([P, 512], F32)
    nc.vector.memset(nd4, 0.0)

    for nb in range(NB):
        for j in range(MC):
            sT = spool.tile([P, NBS], F32)
            for ic in range(HC):
                # scores: stationary xT[:, ic, j-chunk], stream GT
                for s in range(NBS // 512):
                    nc.tensor.matmul(
                        sT[:, s * 512:(s + 1) * 512],
                        xT[:, ic, j * 128:(j + 1) * 128],
                        GT[:, ic, nb * NBS + s * 512: nb * NBS + (s + 1) * 512],
                        start=(ic == 0),
                        stop=(ic == HC - 1),
                    )
            p = ppool.tile([P, NBS], BF16)
            nc.scalar.activation(p, sT, EXP, bias=UBf[:, j:j + 1], scale=0.0625)
            for s in range(NBS // 512):
                base = 32 * (nb * 2 + s)
                nc.tensor.matmul(
                    nd4[base:base + 2, :],
                    UBb[:, 0::2, j],
                    p[:, s * 512:(s + 1) * 512],
                    start=(j == 0),
                    stop=(j == MC - 1),
                    tile_position=(0, base),
                )

    # evacuate packed [numer; denom] (host divides + reassembles)
    ob = fin.tile([P, 512], F32)
    nc.vector.tensor_copy(ob, nd4)
    nc.sync.dma_start(out_ap, ob)


def build_program():
    nc = bacc.Bacc("TRN2", target_bir_lowering=False, debug=False)
    x_ap = nc.dram_tensor("x", [H, N], BF16, kind="ExternalInput").ap()
    a_ap = nc.dram_tensor("wa", [H, H], BF16, kind="ExternalInput").ap()
    w2_ap = nc.dram_tensor("w2", [H, 2], BF16, kind="ExternalInput").ap()
    out_ap = nc.dram_tensor("out", [P, 512], F32, kind="ExternalOutput").ap()
    with tile.TileContext(nc) as tc:
        with ExitStack() as ctx:
            _kernel_body(ctx, tc, out_ap, x_ap, a_ap, w2_ap)
    nc.compile()
    return nc


_PROGRAM = None


def _get_program():
    global _PROGRAM
    if _PROGRAM is None:
        _PROGRAM = build_program()
    return _PROGRAM


def host_fold(Wq, bq, Wk, bk, Wv, bv, Ww, bw):
    """Fold the projection weights (f64 accumulate, f32 store)."""
    A = (Wq.astype(np.float64) @ Wk.astype(np.float64).T).astype(np.float32)
    b16 = ((Wk.astype(np.float64) @ bq.astype(np.float64)) / 16.0).astype(np.float32)
    wvp = (Wv.astype(np.float64) @ Ww.astype(np.float64)[:, 0]).astype(np.float32)
    W2 = np.stack([wvp, b16], axis=1).astype(np.float32)  # [H, 2]
    bw_eff = np.float32(bw[0] + bv.astype(np.float64) @ Ww.astype(np.float64)[:, 0])
    return A, W2, bw_eff


def run(x, Wq, bq, Wk, bk, Wv, bv, Ww, bw, trace=False):
    """Returns (out [8, N], BassKernelResults)."""
    x = np.asarray(x, dtype=np.float32)
    A, W2, bw_eff = host_fold(
        np.asarray(Wq), np.asarray(bq), np.asarray(Wk), np.asarray(bk),
        np.asarray(Wv), np.asarray(bv), np.asarray(Ww), np.asarray(bw),
    )
    # pre-transpose per batch: [B, N, H] -> [B, H, N] (bf16)
    x16 = np.ascontiguousarray(x.astype(ml_dtypes.bfloat16).transpose(0, 2, 1))
    A16 = np.ascontiguousarray(A.astype(ml_dtypes.bfloat16))
    W216 = np.ascontiguousarray(W2.astype(ml_dtypes.bfloat16))

    nc = _get_program()
    in_maps = [
        {"x": x16[b], "wa": A16, "w2": W216}
        for b in range(NCORES)
    ]
    res = run_bass_kernel_spmd(nc, in_maps, list(range(NCORES)), trace=trace)

    def _final(o):
        numer = np.concatenate([o[0], o[32], o[64], o[96]])
        denom = np.concatenate([o[1], o[33], o[65], o[97]])
        return numer / denom + bw_eff

    out = np.stack([_final(res.results[b]["out"]) for b in range(NCORES)], axis=0)
    return out.astype(np.float32), res


def kernel(x, Wq, bq, Wk, bk, Wv, bv, Ww, bw):
    out, _ = run(x, Wq, bq, Wk, bk, Wv, bv, Ww, bw)
    return out


if __name__ == "__main__":
    rng = np.random.default_rng(0)
    s = 1.0 / np.sqrt(H)
    inputs = {
        "x": rng.standard_normal((8, N, H), dtype=np.float32),
        "Wq": rng.uniform(-s, s, (H, H)).astype(np.float32),
        "bq": rng.uniform(-s, s, (H,)).astype(np.float32),
        "Wk": rng.uniform(-s, s, (H, H)).astype(np.float32),
        "bk": rng.uniform(-s, s, (H,)).astype(np.float32),
        "Wv": rng.uniform(-s, s, (H, H)).astype(np.float32),
        "bv": rng.uniform(-s, s, (H,)).astype(np.float32),
        "Ww": rng.uniform(-s, s, (H, 1)).astype(np.float32),
        "bw": rng.uniform(-s, s, (1,)).astype(np.float32),
    }
    out = kernel(**inputs)
    print("kernel out:", out.shape, out.dtype, out[0, :4])


# revision 19
# speedup vs baseline: 1.2417x; 1.0535x over previous
"""Trainium2 Bass kernel for CAAN cross-asset attention.

Reference computation (per batch b of 8):
    q = x @ Wq + bq ; k = x @ Wk + bk ; v = x @ Wv + bv
    beta = softmax((q @ k^T) / 16, axis=-1)          # (N, N), N=2048
    out  = (beta @ v) @ Ww + bw                      # (N,)

Algebraic folding used here (exact up to fp error; ~1e-3 rel err with bf16
matmul operands):
    A   = Wq @ Wk^T                      (host fold, f64->f32)
    s[n,m]  = x_n A x_m^T + row_const[n] + x_m.(Wk bq)
    row-constant terms cancel in softmax, so with
    G = x @ A,  beta16[m] = x_m.(Wk bq)/16,  u0[m] = x_m.(Wv Ww):
    p[n,m]  = exp((G_n . x_m)/16 + beta16[m])
    out[n]  = (sum_m p[n,m] u0[m]) / (sum_m p[n,m]) + (bw + bv.Ww)

Device kernel (SPMD, 1 batch element per NeuronCore, 8 cores):
    - inputs passed as bf16 from host; xT loaded via DMA-transpose (xbar)
    - GT = A-projection of xT (TensorE, bf16 in / f32 PSUM accum)
    - main loop per 1024-wide n-block, per 128-wide m-chunk:
        sT = xT_chunk^T @ GT        (scores, transposed layout, f32 PSUM)
        p  = exp(sT/16 + beta16)    (ScalarE, per-partition bias, bf16 out)
        [numer; denom] += [u0,1]^T @ p   (TensorE, M=2, f32 PSUM accum)
      (u0/beta16 column pairs are computed inline during the first n-block,
       reusing the score matmuls' stationary xT slices)
    - DMA [numer; denom] out; final tiny divide + bw_eff on host
"""

import numpy as np
import ml_dtypes
from contextlib import ExitStack

import concourse.bass as bass
import concourse.tile as tile
from concourse import bacc, mybir
from concourse.bass_utils import run_bass_kernel_spmd

N = 2048          # assets per batch element
H = 256           # hidden
NCORES = 8
P = 128           # partitions
HC = H // P       # h chunks (2)
MC = N // P       # m chunks (16)
NBS = 1024        # n block size
NB = N // NBS     # n blocks (2)

F32 = mybir.dt.float32
BF16 = mybir.dt.bfloat16
EXP = mybir.ActivationFunctionType.Exp


def _kernel_body(ctx: ExitStack, tc: "tile.TileContext", out_ap, x_ap, a_ap, w2_ap):
    nc = tc.nc

    singles = ctx.enter_context(tc.tile_pool(name="singles", bufs=1))

    # xT[p, hc, n] = x[n, hc*128+p]; host supplies x already transposed
    # as [H, N] bf16, so this is a plain fast DMA (chunked so the first GT
    # matmul can start after the first 512 columns land).
    xT = singles.tile([P, HC, N], BF16)
    a_sb = singles.tile([P, HC, H], BF16)
    w2_sb = singles.tile([P, HC, 2], BF16)
    x_r = x_ap.rearrange("(c p) n -> p c n", p=P)
    nc.sync.dma_start(out=xT[:, :, 0:512], in_=x_r[:, :, 0:512])
    # A weights: a_sb[p, ic, o] = A[ic*128+p, o]
    nc.sync.dma_start(out=a_sb, in_=a_ap.rearrange("(c p) o -> p c o", p=P))
    # W2 = [Wv@Ww | (Wk@bq)/16]: w2_sb[p, ic, c]
    nc.sync.dma_start(out=w2_sb, in_=w2_ap.rearrange("(c p) o -> p c o", p=P))
    for q in range(1, 4):
        nc.sync.dma_start(out=xT[:, :, q * 512:(q + 1) * 512], in_=x_r[:, :, q * 512:(q + 1) * 512])

    # GT[p, oc, n] = (x@A)[n, oc*128+p]
    GT = singles.tile([P, HC, N], BF16)
    # UBb[p, c, j]: c=0 -> u0 (bf16), c=2 -> ones; UBf[p, j]: beta16 (f32 bias)
    UBb = singles.tile([P, 3, MC], BF16)
    UBf = singles.tile([P, MC], F32)
    nc.vector.memset(UBb[:, 2, :], 1.0)

    # ---- Phase 1: GT projection + UB columns (pools released before main loop) ----
    with (
        tc.tile_pool(name="gp", bufs=1, space="PSUM") as gpp,
        tc.tile_pool(name="up", bufs=4, space="PSUM") as upp,
    ):
        for oc in range(HC):
            gs = [gpp.tile([P, 512], F32, name=f"g{ns}", tag=f"g{ns}") for ns in range(4)]
            for ic in range(HC):
                for ns in range(4):
                    nc.tensor.matmul(
                        gs[ns],
                        a_sb[:, ic, oc * 128:(oc + 1) * 128],
                        xT[:, ic, ns * 512:(ns + 1) * 512],
                        start=(ic == 0),
                        stop=(ic == HC - 1),
                    )
            for ns in range(4):
                nc.vector.tensor_copy(GT[:, oc, ns * 512:(ns + 1) * 512], gs[ns])
        for j in range(MC):
            ub = upp.tile([P, 2], F32)
            for ic in range(HC):
                nc.tensor.matmul(
                    ub,
                    xT[:, ic, j * 128:(j + 1) * 128],
                    w2_sb[:, ic, :],
                    start=(ic == 0),
                    stop=(ic == HC - 1),
                )
            nc.vector.tensor_copy(UBb[:, 0:1, j], ub[:, 0:1])
            nc.vector.tensor_copy(UBf[:, j:j + 1], ub[:, 1:2])

    # ---- Phase 2: main attention loop ----
    ppool = ctx.enter_context(tc.tile_pool(name="pexp", bufs=4))
    spool = ctx.enter_context(tc.tile_pool(name="spsum", bufs=3, space="PSUM"))
    ndpool = ctx.enter_context(tc.tile_pool(name="ndpsum", bufs=1, space="PSUM"))
    fin = ctx.enter_context(tc.tile_pool(name="fin", bufs=1))

    # [numer; denom] packed into ONE PSUM bank: region (nb, s) sits at
    # partition base 32*(nb*2+s), rows +0 (numer) / +1 (denom), via col-group
    # tile_position. Host reassembles.
    nd4 = ndpool.tile([P, 512], F32)
    nc.vector.memset(nd4, 0.0)

    for nb in range(NB):
        for j in range(MC):
            sT = spool.tile([P, NBS], F32)
            for ic in range(HC):
                # scores: stationary xT[:, ic, j-chunk], stream GT
                for s in range(NBS // 512):
                    nc.tensor.matmul(
                        sT[:, s * 512:(s + 1) * 512],
                        xT[:, ic, j * 128:(j + 1) * 128],
                        GT[:, ic, nb * NBS + s * 512: nb * NBS + (s + 1) * 512],
                        start=(ic == 0),
                        stop=(ic == HC - 1),
                    )
            p = ppool.tile([P, NBS], BF16)
            nc.scalar.activation(p, sT, EXP, bias=UBf[:, j:j + 1], scale=0.0625)
            for s in range(NBS // 512):
                base = 32 * (nb * 2 + s)
                nc.tensor.matmul(
                    nd4[base:base + 2, :],
                    UBb[:, 0::2, j],
                    p[:, s * 512:(s + 1) * 512],
                    start=(j == 0),
                    stop=(j == MC - 1),
                    tile_position=(0, base),
                )

    # evacuate packed [numer; denom] (host divides + reassembles)
    ob = fin.tile([P, 512], F32)
    nc.vector.tensor_copy(ob, nd4)
    nc.sync.dma_start(out_ap, ob)


def build_program():
    nc = bacc.Bacc("TRN2", target_bir_lowering=False, debug=False)
    x_ap = nc.dram_tensor("x", [H, N], BF16, kind="ExternalInput").ap()
    a_ap = nc.dram_tensor("wa", [H, H], BF16, kind="ExternalInput").ap()
    w2_ap = nc.dram_tensor("w2", [H, 2], BF16, kind="ExternalInput").ap()
    out_ap = nc.dram_tensor("out", [P, 512], F32, kind="ExternalOutput").ap()
    with tile.TileContext(nc) as tc:
        with ExitStack() as ctx:
            _kernel_body(ctx, tc, out_ap, x_ap, a_ap, w2_ap)
    nc.compile()
    return nc


_PROGRAM = None


def _get_program():
    global _PROGRAM
    if _PROGRAM is None:
        _PROGRAM = build_program()
    return _PROGRAM


def host_fold(Wq, bq, Wk, bk, Wv, bv, Ww, bw):
    """Fold the projection weights (f64 accumulate, f32 store)."""
    A = (Wq.astype(np.float64) @ Wk.astype(np.float64).T).astype(np.float32)
    b16 = ((Wk.astype(np.float64) @ bq.astype(np.float64)) / 16.0).astype(np.float32)
    wvp = (Wv.astype(np.float64) @ Ww.astype(np.float64)[:, 0]).astype(np.float32)
    W2 = np.stack([wvp, b16], axis=1).astype(np.float32)  # [H, 2]
    bw_eff = np.float32(bw[0] + bv.astype(np.float64) @ Ww.astype(np.float64)[:, 0])
    return A, W2, bw_eff


def run(x, Wq, bq, Wk, bk, Wv, bv, Ww, bw, trace=False):
    """Returns (out [8, N], BassKernelResults)."""
    x = np.asarray(x, dtype=np.float32)
    A, W2, bw_eff = host_fold(
        np.asarray(Wq), np.asarray(bq), np.asarray(Wk), np.asarray(bk),
        np.asarray(Wv), np.asarray(bv), np.asarray(Ww), np.asarray(bw),
    )
    # pre-transpose per batch: [B, N, H] -> [B, H, N] (bf16)
    x16 = np.ascontiguousarray(x.astype(ml_dtypes.bfloat16).transpose(0, 2, 1))
    A16 = np.ascontiguousarray(A.astype(ml_dtypes.bfloat16))
    W216 = np.ascontiguousarray(W2.astype(ml_dtypes.bfloat16))

    nc = _get_program()
    in_maps = [
        {"x": x16[b], "wa": A16, "w2": W216}
        for b in range(NCORES)
    ]
    last_err = None
    for attempt in range(3):
        try:
            res = run_bass_kernel_spmd(nc, in_maps, list(range(NCORES)), trace=trace)
            break
        except Exception as e:  # transient NRT device wedges have been observed
            last_err = e
            if attempt == 2:
                raise
            import time as _time
            _time.sleep(20 * (attempt + 1))

    def _final(o):
        numer = np.concatenate([o[0], o[32], o[64], o[96]])
        denom = np.concatenate([o[1], o[33], o[65], o[97]])
        return numer / denom + bw_eff

    out = np.stack([_final(res.results[b]["out"]) for b in range(NCORES)], axis=0)
    return out.astype(np.float32), res


def kernel(x, Wq, bq, Wk, bk, Wv, bv, Ww, bw):
    out, _ = run(x, Wq, bq, Wk, bk, Wv, bv, Ww, bw)
    return out


if __name__ == "__main__":
    rng = np.random.default_rng(0)
    s = 1.0 / np.sqrt(H)
    inputs = {
        "x": rng.standard_normal((8, N, H), dtype=np.float32),
        "Wq": rng.uniform(-s, s, (H, H)).astype(np.float32),
        "bq": rng.uniform(-s, s, (H,)).astype(np.float32),
        "Wk": rng.uniform(-s, s, (H, H)).astype(np.float32),
        "bk": rng.uniform(-s, s, (H,)).astype(np.float32),
        "Wv": rng.uniform(-s, s, (H, H)).astype(np.float32),
        "bv": rng.uniform(-s, s, (H,)).astype(np.float32),
        "Ww": rng.uniform(-s, s, (H, 1)).astype(np.float32),
        "bw": rng.uniform(-s, s, (1,)).astype(np.float32),
    }
    out = kernel(**inputs)
    print("kernel out:", out.shape, out.dtype, out[0, :4])
